# revision 39
# baseline (speedup 1.0000x reference)
"""Graph-transformer encoder kernel for trn2, 8-core SPMD.

Sharding: nodes split across cores (1250/core, padded to 1280 = 10 tiles
of 128). Weights replicated (bf16). Per layer: local Q/K/V matmuls, K/V
all-gathered (concat [*,512] rows) into a shared DRAM table. The
all-neighbor v-sum (the softmax's uniform tail over the 24 non-top-8
neighbors) is computed as a dense matmul against a host-precomputed
block count matrix M ([dst-tile 10][src-tile 80] blocks of 128x128,
bf16): sall = sum_s M[s,t].T @ v_s. This streams 26MB/layer of M
sequentially instead of issuing 40K random 512B HBM gather descriptors
(latency-bound at ~180ns each). Only the top-8 k|v rows are fetched
with dma_gather; attention math on DVE/ACT in bf16/f32.
"""
import sys
sys.path.insert(0, '/opt/trn_rl_repo')
import numpy as np
import concourse.bass as bass
import concourse.bacc as bacc
import concourse.mybir as mybir
import concourse.tile as tile
from concourse import bass_utils
from concourse.masks import make_identity

f32 = mybir.dt.float32
bf16 = mybir.dt.bfloat16
i32 = mybir.dt.int32
i16 = mybir.dt.int16
u32 = mybir.dt.uint32
AF = mybir.ActivationFunctionType
OP = mybir.AluOpType
AX = mybir.AxisListType

HID, D, H, DH, K, L, F = 256, 32, 8, 32, 8, 2, 2
RSQ_DH = 1.0 / np.sqrt(np.float32(DH))
EPS_LN = 1e-5
MAGIC = 0x5f3759df


def build(NCORE, T, REPEAT=1, skip_cc=False, skip_gather=False):
    """Build the Bass program for NCORE cores, T 128-node tiles per core."""
    NPAD = 128 * T
    NTAB = NCORE * NPAD
    NCH = (NCORE * T) // 16  # src-tile chunks for the M-matmul

    nc = bacc.Bacc("TRN2", target_bir_lowering=False, debug=False,
                   enable_asserts=True, num_devices=NCORE)

    h_in = nc.dram_tensor("h_in", [NPAD, HID], f32, kind="ExternalInput")
    edge_in = nc.dram_tensor("edge_in", [128, T * D], f32, kind="ExternalInput")
    agf_in = nc.dram_tensor("agf_in", [128, T * D], f32, kind="ExternalInput")
    m_in = nc.dram_tensor("m_in", [T * NCH, 128, 16 * 128], bf16, kind="ExternalInput")
    wT_in = nc.dram_tensor("wT_in", [2 * 6, HID, HID], bf16, kind="ExternalInput")
    bias_in = nc.dram_tensor("bias_in", [1, 12 * HID], bf16, kind="ExternalInput")
    gam_in = nc.dram_tensor("gam_in", [128, HID], f32, kind="ExternalInput")
    bet_in = nc.dram_tensor("bet_in", [128, HID], f32, kind="ExternalInput")
    out_dram = nc.dram_tensor("out", [NPAD, HID], f32, kind="ExternalOutput")
    kvdt = bf16
    kv_all = nc.dram_tensor("kv_all", [NTAB, 2 * HID], kvdt, kind="Internal",
                            addr_space="Shared" if NCORE > 4 else "Local")

    with tile.TileContext(nc) as tc:
        with tc.tile_pool(name="wp", bufs=1) as wp, \
             tc.tile_pool(name="sp", bufs=2) as sp, \
             tc.tile_pool(name="hp", bufs=3) as hp, \
             tc.tile_pool(name="big", bufs=3) as big, \
             tc.tile_pool(name="qp", bufs=1) as qp, \
             tc.tile_pool(name="pp", bufs=2, space="PSUM") as pp, \
             tc.tile_pool(name="dram", bufs=1, space="DRAM") as dram:

            # ---------------- loads ----------------
            h_in_t = h_in.ap().rearrange("(t p) c -> p t c", p=128)
            edge_sb = qp.tile([128, T, D], f32)
            nc.sync.dma_start(edge_sb[:], edge_in.ap().rearrange("p (t d) -> p t d", d=D))
            agf_sb = qp.tile([128, T, D], f32)
            nc.sync.dma_start(agf_sb[:], agf_in.ap().rearrange("p (t d) -> p t d", d=D))
            wT_sb = wp.tile([128, 12, 2, HID], bf16)
            nc.sync.dma_start(wT_sb[:], wT_in.ap().rearrange("w (kb p) n -> p w kb n", p=128))
            bias_sb = wp.tile([1, 12 * HID], bf16)
            nc.sync.dma_start(bias_sb[:], bias_in.ap())
            gam_sb = wp.tile([128, HID], f32)
            nc.sync.dma_start(gam_sb[:], gam_in.ap())
            bet_sb = wp.tile([128, HID], f32)
            nc.sync.dma_start(bet_sb[:], bet_in.ap())

            ident = wp.tile([128, 128], f32)
            make_identity(nc, ident[:])
            ones1 = wp.tile([1, 128], bf16)
            nc.gpsimd.memset(ones1[:], 1.0)
            iota32 = wp.tile([128, D], i32)
            nc.gpsimd.iota(iota32[:], pattern=[[1, D]], base=0, channel_multiplier=0)
            iotaf = wp.tile([128, D], f32)
            nc.vector.tensor_copy(iotaf[:], iota32[:])

            kv_loc = dram.tile([NPAD, 2 * HID], kvdt)

            # ---------------- helpers ----------------
            def rsqrt_newton(w_ap, n):
                """1/sqrt(w) for [128, n] f32 AP; returns tile."""
                j = sp.tile([128, n], i32, tag=f"nrj{n}")
                nc.vector.tensor_scalar(j[:], w_ap.bitcast(i32), 1, None,
                                        op0=OP.logical_shift_right)
                k2 = sp.tile([128, n], i32, tag=f"nrk{n}")
                nc.vector.tensor_scalar(k2[:], j[:], -1, MAGIC,
                                        op0=OP.mult, op1=OP.add)
                y = sp.tile([128, n], f32, tag=f"nry{n}")
                nc.vector.tensor_copy(y[:], k2[:].bitcast(f32))
                for it in range(3):
                    a = sp.tile([128, n], f32, tag=f"nra{n}")
                    nc.vector.tensor_tensor(a[:], y[:], y[:], op=OP.mult)
                    b = sp.tile([128, n], f32, tag=f"nrb{n}")
                    nc.vector.tensor_tensor(b[:], a[:], w_ap, op=OP.mult)
                    c = sp.tile([128, n], f32, tag=f"nrc{n}")
                    nc.vector.tensor_scalar(c[:], b[:], -0.5, 1.5,
                                            op0=OP.mult, op1=OP.add)
                    y2 = sp.tile([128, n], f32, tag=f"nry{n}")
                    nc.vector.tensor_tensor(y2[:], y[:], c[:], op=OP.mult)
                    y = y2
                return y

            sink_n = [0]

            def ln_stats(x_ap, s_all, q_all, t, skip_sum=False):
                """accumulate sum and sumsq of x [128,256] into col t."""
                if not skip_sum:
                    sink = sp.tile([128, HID], f32, tag="sink")
                    nc.scalar.activation(sink[:], x_ap, AF.Identity,
                                         accum_out=s_all[:, t:t + 1])
                sink2 = sp.tile([128, HID], f32, tag="sink")
                nc.scalar.activation(sink2[:], x_ap, AF.Square,
                                     accum_out=q_all[:, t:t + 1])

            def ln_coeffs(s_ap, q_ap, n=T):
                mu = sp.tile([128, n], f32, tag=f"lmu{n}")
                nc.vector.tensor_scalar(mu[:], s_ap, 1.0 / HID, None, op0=OP.mult)
                m2 = sp.tile([128, n], f32, tag=f"lm2{n}")
                nc.vector.tensor_tensor(m2[:], mu[:], mu[:], op=OP.mult)
                var = sp.tile([128, n], f32, tag=f"lvar{n}")
                nc.vector.scalar_tensor_tensor(var[:], q_ap, 1.0 / HID, m2[:],
                                               op0=OP.mult, op1=OP.subtract)
                w = sp.tile([128, n], f32, tag=f"lw{n}")
                nc.vector.tensor_scalar(w[:], var[:], EPS_LN, None, op0=OP.add)
                rstd = rsqrt_newton(w[:], n)
                nmr = sp.tile([128, n], f32, tag=f"lnmr{n}")
                nc.vector.scalar_tensor_tensor(nmr[:], mu[:], -1.0, rstd[:],
                                               op0=OP.mult, op1=OP.mult)
                return rstd, nmr

            def ln_apply(x_ap, rstd, nmr, t, resid_ap, out_tile):
                """out = resid + LN(x)*gamma + beta (resid_ap None -> no resid)."""
                xh = sp.tile([128, HID], f32, tag="xh")
                nc.scalar.activation(xh[:], x_ap, AF.Identity,
                                     scale=rstd[:, t:t + 1], bias=nmr[:, t:t + 1])
                xg = sp.tile([128, HID], f32, tag="xg")
                nc.vector.tensor_tensor(xg[:], xh[:], gam_sb[:], op=OP.mult)
                if resid_ap is None:
                    nc.vector.tensor_tensor(out_tile, xg[:], bet_sb[:], op=OP.add)
                else:
                    hb = sp.tile([128, HID], f32, tag="hb")
                    nc.vector.tensor_tensor(hb[:], resid_ap, bet_sb[:], op=OP.add)
                    nc.vector.tensor_tensor(out_tile, xg[:], hb[:], op=OP.add)

            def mish_from_psum(ps_ap, out_tile, accum=None):
                """out = mish(ps) elementwise [128,256]; optional sum accum."""
                u = sp.tile([128, HID], f32, tag="mu_")
                nc.scalar.activation(u[:], ps_ap, AF.Exp)
                dsq = sp.tile([128, HID], f32, tag="mdsq")
                nc.scalar.activation(dsq[:], u[:], AF.Square, bias=1.0)
                nc.vector.tensor_scalar(dsq[:], dsq[:], 1.0, None, op0=OP.add)
                rr = sp.tile([128, HID], f32, tag="mrr")
                nc.vector.reciprocal_approx_fast(rr[:], dsq[:])
                nc.vector.scalar_tensor_tensor(u[:], dsq[:], -2.0, rr[:],
                                               op0=OP.add, op1=OP.mult)
                nc.vector.scalar_tensor_tensor(out_tile, u[:], 1.0, ps_ap,
                                               op0=OP.bypass, op1=OP.mult,
                                               accum_out=accum)

            def transpose_to(src_ap_fn, tag):
                """[128,256] node-major f32 -> [128, 2, 128] chan-major bf16."""
                dst = sp.tile([128, 2, 128], bf16, tag=tag)
                for kb in range(2):
                    tp = pp.tile([128, 128], f32, tag="tp", space="PSUM")
                    nc.tensor.transpose(tp[:], src_ap_fn(kb), ident[:])
                    nc.vector.tensor_copy(dst[:, kb, :], tp[:])
                return dst

            def linear(xT, widx, psum_tile, n0=0, n1=HID):
                """psum = x @ W.T + b via stationary xT chunks."""
                for kb in range(2):
                    nc.tensor.matmul(psum_tile[:, n0:n1], xT[:, kb, :],
                                     wT_sb[:, widx, kb, n0:n1],
                                     start=(kb == 0), stop=False)
                nc.tensor.matmul(psum_tile[:, n0:n1], ones1[:],
                                 bias_sb[:, widx * HID + n0:widx * HID + n1],
                                 start=False, stop=True)

            # ---------------- top-8 precompute ----------------
            wn8_all = wp.tile([128, T, K], f32)
            nid16_all = wp.tile([128, T * K], i16)
            kidxw = wp.tile([128, T * 64], i16)
            for t in range(T):
                ew = edge_sb[:, t, :]
                m8 = sp.tile([128, K], f32, tag="m8")
                nc.vector.max(m8[:], ew)
                pos8 = sp.tile([128, K], u32, tag="pos8")
                nc.vector.max_index(pos8[:], m8[:], ew)
                s8 = sp.tile([128, 1], f32, tag="s8sum")
                nc.vector.tensor_reduce(s8[:], m8[:], axis=AX.X, op=OP.add)
                s8e = sp.tile([128, 1], f32, tag="s8e")
                nc.vector.tensor_scalar(s8e[:], s8[:], 1e-5, None, op0=OP.add)
                rs = sp.tile([128, 1], f32, tag="rs8")
                nc.vector.reciprocal(rs[:], s8e[:])
                nc.vector.scalar_tensor_tensor(
                    wn8_all[:, t, :], m8[:], float(RSQ_DH),
                    rs[:].broadcast_to((128, K)), op0=OP.mult, op1=OP.mult)
                pos8f = sp.tile([128, K], f32, tag="pos8f")
                nc.vector.tensor_copy(pos8f[:], pos8[:])
                oh = sp.tile([128, K, D], f32, tag="oh")
                nc.vector.tensor_tensor(
                    oh[:], pos8f[:].unsqueeze(2).broadcast_to((128, K, D)),
                    iotaf[:].unsqueeze(1).broadcast_to((128, K, D)), op=OP.is_equal)
                ohi = sp.tile([128, K, D], f32, tag="ohi")
                nc.vector.tensor_tensor(
                    ohi[:], oh[:],
                    agf_sb[:, t, :].unsqueeze(1).broadcast_to((128, K, D)), op=OP.mult)
                nid8 = sp.tile([128, K], f32, tag="nid8")
                nc.vector.tensor_reduce(nid8[:], ohi[:], axis=AX.X, op=OP.add)
                nc.vector.tensor_copy(nid16_all[:, t * K:(t + 1) * K], nid8[:])
            # wrap nid16 into gather layout: kidxw[pp,t*64+i*8+ph] = nid16[ph*16+pp%16, t*8+i]
            for ph in range(8):
                src = nid16_all[ph * 16:(ph + 1) * 16, :].rearrange(
                    "p (t i) -> p t i", i=K)
                dst = kidxw[0:16, :].rearrange("p (t i e) -> p t i e", i=K, e=8)[:, :, :, ph]
                nc.sync.dma_start(dst, src)
            nc.sync.dma_start(kidxw[16:32, :], kidxw[0:16, :])
            nc.sync.dma_start(kidxw[32:64, :], kidxw[0:32, :])
            nc.sync.dma_start(kidxw[64:128, :], kidxw[0:64, :])

            # ---------------- initial LN ----------------
            s_all = sp.tile([128, T], f32, tag="lns")
            q_all = sp.tile([128, T], f32, tag="lnq")
            for t in range(T):
                hld = sp.tile([128, HID], f32, tag="hld")
                nc.sync.dma_start(hld[:], h_in_t[:, t, :])
                ln_stats(hld[:], s_all, q_all, t)
            rstd, nmr = ln_coeffs(s_all[:], q_all[:])
            h_cur = []
            for t in range(T):
                hld = sp.tile([128, HID], f32, tag="hld")
                nc.sync.dma_start(hld[:], h_in_t[:, t, :])
                g = hp.tile([128, HID], f32, tag=f"h{t}")
                ln_apply(hld[:], rstd, nmr, t, None, g[:])
                h_cur.append(g)

            # ---------------- layers ----------------
            for l in [ll % L for ll in range(REPEAT * L)]:
                # P1: QKV + kv table
                q_tiles = []
                for t in range(T):
                    hT = transpose_to(lambda kb, t=t: h_cur[t][:, kb * 128:(kb + 1) * 128], "hT")
                    psq = pp.tile([128, HID], f32, tag="psq", space="PSUM")
                    pskv = pp.tile([128, 2 * HID], f32, tag="pskv", space="PSUM")
                    linear(hT, l * 6 + 0, psq)
                    linear(hT, l * 6 + 1, pskv, 0, HID)
                    # shift: write v into second half by using slicing on psum
                    for kb in range(2):
                        nc.tensor.matmul(pskv[:, HID:2 * HID], hT[:, kb, :],
                                         wT_sb[:, l * 6 + 2, kb, :],
                                         start=(kb == 0), stop=False)
                    nc.tensor.matmul(pskv[:, HID:2 * HID], ones1[:],
                                     bias_sb[:, (l * 6 + 2) * HID:(l * 6 + 3) * HID],
                                     start=False, stop=True)
                    kvst = sp.tile([128, 2 * HID], kvdt, tag="kvst")
                    nc.scalar.activation(kvst[:, 0:HID], pskv[:, HID:2 * HID],
                                         AF.Identity)   # v first
                    nc.scalar.activation(kvst[:, HID:2 * HID], pskv[:, 0:HID],
                                         AF.Identity)   # then k
                    nc.sync.dma_start(
                        kv_loc[:].rearrange("(t p) c -> p t c", p=128)[:, t, :], kvst[:])
                    qsb = qp.tile([128, HID], kvdt, tag=f"q{t}")
                    nc.scalar.activation(qsb[:], psq[:], AF.Identity)
                    q_tiles.append(qsb)
                    # all-gather this tile while P1 continues;
                    # out rows [t*1024 + r*128 + p] for core r's rows p
                    if NCORE > 1 and not skip_cc:
                        nc.gpsimd.collective_compute(
                            "AllGather", OP.bypass,
                            replica_groups=[list(range(NCORE))],
                            ins=[kv_loc[t * 128:(t + 1) * 128, :]],
                            outs=[kv_all.ap()[t * NCORE * 128:(t + 1) * NCORE * 128, :]])
                    else:
                        nc.sync.dma_start(
                            kv_all.ap()[t * NCORE * 128:t * NCORE * 128 + 128, :],
                            kv_loc[t * 128:(t + 1) * 128, :])

                # P2a: sall via M-matmul over the kv table's v half.
                # sall[t] = sum_s M[s,t].T @ v_s, accumulated chunk-by-chunk.
                sacc = [qp.tile([128, HID], f32, tag=f"sa{t}", name=f"sa{t}")
                        for t in range(T)]
                for c in range(NCH):
                    vts = big.tile([128, 16, HID], kvdt, tag="vsall")
                    nc.sync.dma_start(
                        vts[:],
                        kv_all.ap()[c * 16 * 128:(c + 1) * 16 * 128, 0:HID]
                        .rearrange("(s p) e -> p s e", p=128))
                    for t in range(T):
                        mtile = big.tile([128, 16, 128], bf16, tag="mt")
                        nc.sync.dma_start(
                            mtile[:],
                            m_in.ap()[t * NCH + c].rearrange("p (s i) -> p s i", i=128))
                        psal = pp.tile([128, HID], f32, tag="psq", space="PSUM")
                        for si in range(16):
                            nc.tensor.matmul(psal[:], mtile[:, si, :], vts[:, si, :],
                                             start=(si == 0), stop=(si == 15))
                        if c == 0:
                            nc.vector.tensor_copy(sacc[t][:], psal[:])
                        else:
                            nc.vector.tensor_tensor(sacc[t][:], sacc[t][:], psal[:],
                                                    op=OP.add)

                # P2b: per tile, fused attention + Wo + mish + LN + FFN + LN
                ms_all = sp.tile([128, T], f32, tag="lns")
                mq_all = sp.tile([128, T], f32, tag="lnq")
                fs_all = sp.tile([128, T], f32, tag="fns")
                fq_all = sp.tile([128, T], f32, tag="fnq")
                h_next = []
                for t in range(T):
                    sall = sacc[t]

                    kv8 = big.tile([128, K, 2 * HID], kvdt, tag="kv8")
                    if skip_gather:
                        nc.gpsimd.memset(kv8[:], 0.1)
                    else:
                        nc.gpsimd.dma_gather(
                            out_ap=kv8[:], in_ap=kv_all.ap(),
                            idxs_ap=kidxw[:, t * 64:(t + 1) * 64],
                            num_idxs=1024, num_idxs_reg=1024, elem_size=2 * HID,
                            single_packet=False)

                    # scores on top-8
                    t8 = big.tile([128, K, HID], kvdt, tag="t8wv")
                    nc.vector.tensor_tensor(
                        t8[:], kv8[:, :, HID:2 * HID],
                        q_tiles[t][:].unsqueeze(1).broadcast_to((128, K, HID)),
                        op=OP.mult)
                    # per-(d,h) dot: fold DH=32 by contiguous tree halving
                    t4 = t8[:].rearrange("p d (h e) -> p d h e", e=DH)
                    for lv in (16, 8):
                        nc.vector.tensor_tensor(t4[:, :, :, 0:lv], t4[:, :, :, 0:lv],
                                                t4[:, :, :, lv:2 * lv], op=OP.add)
                    s8t = sp.tile([128, K, H], f32, tag="s8t")
                    nc.vector.tensor_reduce(
                        s8t[:], t4[:, :, :, 0:8], axis=AX.X, op=OP.add)
                    l8 = sp.tile([128, K, H], f32, tag="l8")
                    nc.vector.tensor_tensor(
                        l8[:], s8t[:],
                        wn8_all[:, t, :].unsqueeze(2).broadcast_to((128, K, H)),
                        op=OP.mult)
                    # logits here are O(0.3): softmax needs no max-shift, so
                    # Z = 24 + sum(exp(l8)), c0 = 1/Z, d8 = (exp(l8)-1)/Z
                    e8 = sp.tile([128, K, H], f32, tag="e8")
                    nc.scalar.activation(e8[:], l8[:], AF.Exp)
                    zs = sp.tile([128, H], f32, tag="zs")
                    nc.vector.tensor_reduce(zs[:], e8[:].transpose((0, 2, 1)),
                                            axis=AX.X, op=OP.add)
                    Z = sp.tile([128, H], f32, tag="Z")
                    nc.vector.tensor_scalar(Z[:], zs[:], float(D - K), None,
                                            op0=OP.add)
                    c0 = sp.tile([128, H], f32, tag="c0")
                    nc.vector.reciprocal_approx_fast(c0[:], Z[:])
                    d8 = sp.tile([128, K, H], kvdt, tag="d8c")
                    nc.vector.scalar_tensor_tensor(
                        d8[:], e8[:], -1.0,
                        c0[:].unsqueeze(1).broadcast_to((128, K, H)),
                        op0=OP.add, op1=OP.mult)
                    wv = big.tile([128, K, HID], kvdt, tag="t8wv")
                    nc.vector.tensor_tensor(
                        wv[:].rearrange("p d (h e) -> p d h e", e=DH),
                        kv8[:, :, 0:HID].rearrange("p d (h e) -> p d h e", e=DH),
                        d8[:].unsqueeze(3).broadcast_to((128, K, H, DH)),
                        op=OP.mult)
                    # fold K=8 neighbors by contiguous tree halving
                    for lv in (4, 2):
                        nc.vector.tensor_tensor(wv[:, 0:lv, :], wv[:, 0:lv, :],
                                                wv[:, lv:2 * lv, :], op=OP.add)
                    wsum = sp.tile([128, HID], f32, tag="wsum")
                    nc.vector.tensor_tensor(wsum[:], wv[:, 0, :], wv[:, 1, :],
                                            op=OP.add)
                    cs = sp.tile([128, HID], f32, tag="cs")
                    nc.vector.tensor_tensor(
                        cs[:].rearrange("p (h e) -> p h e", e=DH),
                        sall[:].rearrange("p (h e) -> p h e", e=DH),
                        c0[:].unsqueeze(2).broadcast_to((128, H, DH)), op=OP.mult)
                    o_sb = sp.tile([128, HID], f32, tag="osb")
                    nc.vector.tensor_tensor(o_sb[:], wsum[:], cs[:], op=OP.add)

                    # Wo + mish
                    oT = transpose_to(lambda kb, o=o_sb: o[:, kb * 128:(kb + 1) * 128], "oT")
                    psmo = pp.tile([128, HID], f32, tag="pso", space="PSUM")
                    linear(oT, l * 6 + 3, psmo)
                    mo = qp.tile([128, HID], f32, tag=f"mo{t}")
                    mish_from_psum(psmo[:], mo[:], accum=ms_all[:, t:t + 1])
                    ln_stats(mo[:], ms_all, mq_all, t, skip_sum=True)
                    rstd1, nmr1 = ln_coeffs(ms_all[:, t:t + 1], mq_all[:, t:t + 1], 1)
                    h1 = hp.tile([128, HID], f32, tag=f"h{t}", name=f"h1_{t}")
                    ln_apply(mo[:], rstd1, nmr1, 0, h_cur[t][:], h1[:])

                    # FFN, fused per tile
                    xT = transpose_to(lambda kb, hh=h1: hh[:, kb * 128:(kb + 1) * 128], "hT")
                    psf1 = pp.tile([128, HID], f32, tag="pso", space="PSUM")
                    linear(xT, l * 6 + 4, psf1)
                    f1 = sp.tile([128, HID], f32, tag="f1")
                    mish_from_psum(psf1[:], f1[:])
                    f1T = transpose_to(lambda kb, f=f1: f[:, kb * 128:(kb + 1) * 128], "oT")
                    psf2 = pp.tile([128, HID], f32, tag="pso", space="PSUM")
                    linear(f1T, l * 6 + 5, psf2)
                    f2 = qp.tile([128, HID], f32, tag=f"mo{t}", name=f"f2_{t}")
                    mish_from_psum(psf2[:], f2[:], accum=fs_all[:, t:t + 1])
                    ln_stats(f2[:], fs_all, fq_all, t, skip_sum=True)
                    rstd2, nmr2 = ln_coeffs(fs_all[:, t:t + 1], fq_all[:, t:t + 1], 1)
                    h2 = hp.tile([128, HID], f32, tag=f"h{t}", name=f"h2_{t}")
                    ln_apply(f2[:], rstd2, nmr2, 0, h1[:], h2[:])
                    h_next.append(h2)
                h_cur = h_next

            # ---------------- final LN ----------------
            s_all = sp.tile([128, T], f32, tag="lns")
            q_all = sp.tile([128, T], f32, tag="lnq")
            for t in range(T):
                ln_stats(h_cur[t][:], s_all, q_all, t)
            rstd, nmr = ln_coeffs(s_all[:], q_all[:])
            for t in range(T):
                ot = sp.tile([128, HID], f32, tag="otile")
                ln_apply(h_cur[t][:], rstd, nmr, t, None, ot[:])
                nc.sync.dma_start(
                    out_dram.ap().rearrange("(t p) c -> p t c", p=128)[:, t, :], ot[:])

    nc.compile()
    return nc


# ---------------- host-side marshalling ----------------

def wrap_idx(flat):
    """flat [M] -> wrapped [128, M//16] int16 (replicated across 8 groups)."""
    M = flat.shape[0]
    w = np.empty((128, M // 16), np.int16)
    blk = flat.reshape(M // 16, 16).T.astype(np.int16)
    for g in range(8):
        w[g * 16:(g + 1) * 16, :] = blk
    return w


def make_in_maps(inputs, NCORE, NLOC, T):
    import ml_dtypes
    bfnp = ml_dtypes.bfloat16
    NPAD = 128 * T
    NCH = (NCORE * T) // 16
    h = np.asarray(inputs["h"], np.float32)
    neigh = np.asarray(inputs["neigh_idx"]).astype(np.int64)
    ew = np.asarray(inputs["edge_w"], np.float32)
    Wq, bq = np.asarray(inputs["Wq"], np.float32), np.asarray(inputs["bq"], np.float32)
    Wk, bk = np.asarray(inputs["Wk"], np.float32), np.asarray(inputs["bk"], np.float32)
    Wv, bv = np.asarray(inputs["Wv"], np.float32), np.asarray(inputs["bv"], np.float32)
    Wo, bo = np.asarray(inputs["Wo"], np.float32), np.asarray(inputs["bo"], np.float32)
    Wf, bf = np.asarray(inputs["Wf"], np.float32), np.asarray(inputs["bf"], np.float32)
    gamma = np.asarray(inputs["gamma"], np.float32)
    beta = np.asarray(inputs["beta"], np.float32)

    wT = np.stack([w.T.copy() for l in range(L) for w in
                   (Wq[l], Wk[l], Wv[l], Wo[l], Wf[l, 0], Wf[l, 1])]).astype(bfnp)
    bias = np.concatenate([b for l in range(L) for b in
                           (bq[l], bk[l], bv[l], bo[l], bf[l, 0], bf[l, 1])]
                          )[None, :].astype(bfnp)
    gam_rep = np.tile(gamma[None, :], (128, 1)).copy()
    bet_rep = np.tile(beta[None, :], (128, 1)).copy()

    in_maps = []
    for r in range(NCORE):
        sl = slice(r * NLOC, (r + 1) * NLOC)
        h_loc = np.zeros((NPAD, HID), np.float32)
        h_loc[:NLOC] = h[sl]
        ew_loc = np.zeros((NPAD, D), np.float32)
        ew_loc[:NLOC] = ew[sl]
        ng = np.zeros((NPAD, D), np.int64)
        ng[:NLOC] = neigh[sl]
        # chunked-AG table rows: chunk t = tile t of every core,
        # laid out [t*1024 + r*128 + p]
        rr = ng // NLOC
        jj = ng % NLOC
        tt = jj // 128
        pp = jj % 128
        ag = tt * (NCORE * 128) + rr * 128 + pp

        e3 = ew_loc.reshape(T, 128, D).transpose(1, 0, 2).reshape(128, T * D)
        a3 = ag.reshape(T, 128, D).transpose(1, 0, 2).reshape(128, T * D).astype(np.float32)
        # M block counts: m9[t*NCH + s//16, q, (s%16)*128 + i] = #edges
        # (dst local node t*128+i) <- (src table row s*128+q)
        s_glob = (ag // 128).astype(np.int64)
        q_glob = (ag % 128).astype(np.int64)
        rows = np.arange(NPAD)
        t_ = rows // 128
        i_ = rows % 128
        m9 = np.zeros((T * NCH, 128, 16 * 128), np.float32)
        idx0 = t_[:, None] * NCH + s_glob // 16
        idx2 = (s_glob % 16) * 128 + i_[:, None]
        np.add.at(m9, (idx0, q_glob, idx2), 1.0)
        in_maps.append({
            "h_in": h_loc, "edge_in": np.ascontiguousarray(e3),
            "agf_in": np.ascontiguousarray(a3), "m_in": m9.astype(bfnp),
            "wT_in": wT, "bias_in": bias, "gam_in": gam_rep, "bet_in": bet_rep,
        })
    return in_maps


def assemble(results, NCORE, NLOC, T):
    return np.concatenate([results[r]["out"][:NLOC] for r in range(NCORE)], axis=0)


# ---------------- persistent PJRT runner ----------------
import jax
from jax.sharding import Mesh, PartitionSpec
from jax.experimental.shard_map import shard_map
from concourse import bass2jax


class Runner:
    def __init__(self, nc, n_cores):
        bass2jax.install_neuronx_cc_hook()
        self.nc = nc
        self.n_cores = n_cores
        in_names, out_names, out_avals, zero_outs = [], [], [], []
        for alloc in nc.m.functions[0].allocations:
            if not isinstance(alloc, mybir.MemoryLocationSet):
                continue
            name = alloc.memorylocations[0].name
            if alloc.kind == "ExternalInput":
                if nc.partition_id_tensor is None or name != nc.partition_id_tensor.name:
                    in_names.append(name)
            elif alloc.kind == "ExternalOutput":
                shape = tuple(alloc.tensor_shape)
                dtype = mybir.dt.np(alloc.dtype)
                out_names.append(name)
                out_avals.append(jax.core.ShapedArray(shape, dtype))
                zero_outs.append(np.zeros(shape, dtype))
        self.in_names, self.out_names = list(in_names), out_names
        self.out_avals, self.zero_outs = out_avals, zero_outs
        n_params = len(in_names)
        pname = nc.partition_id_tensor.name if nc.partition_id_tensor else None
        all_names = in_names + out_names + ([pname] if pname else [])

        def _body(*args):
            operands = list(args)
            if pname:
                operands.append(bass2jax.partition_id_tensor())
            outs = bass2jax._bass_exec_p.bind(
                *operands, out_avals=tuple(out_avals), in_names=tuple(all_names),
                out_names=tuple(out_names), lowering_input_output_aliases=(),
                sim_require_finite=True, sim_require_nnan=True, nc=nc)
            return tuple(outs)

        devices = jax.devices()[:n_cores]
        mesh = Mesh(np.asarray(devices), ("core",))
        in_specs = (PartitionSpec("core"),) * (n_params + len(out_names))
        out_specs = (PartitionSpec("core"),) * len(out_names)
        self.fn = jax.jit(shard_map(_body, mesh=mesh, in_specs=in_specs,
                                    out_specs=out_specs, check_rep=False),
                          keep_unused=True)
        self._cached_dev_inputs = None

    def prepare(self, in_maps):
        concat = [np.concatenate([np.asarray(in_maps[c][n]) for c in range(self.n_cores)],
                                 axis=0) for n in self.in_names]
        concat += [np.zeros((self.n_cores * z.shape[0], *z.shape[1:]), z.dtype)
                   for z in self.zero_outs]
        self._cached_dev_inputs = [jax.device_put(a) for a in concat]
        for a in self._cached_dev_inputs:
            a.block_until_ready()

    def run(self):
        outs = self.fn(*self._cached_dev_inputs)
        for o in outs:
            o.block_until_ready()
        return outs

    def results(self, outs):
        res = []
        for c in range(self.n_cores):
            d = {}
            for i, n in enumerate(self.out_names):
                d[n] = np.asarray(outs[i]).reshape(
                    self.n_cores, *self.out_avals[i].shape)[c]
            res.append(d)
        return res


# ---------------- harness entry point ----------------
_STATE = {}

NCORE_RUN, T_RUN, NLOC_RUN = 8, 10, 1250


def kernel(**inputs):
    """Full-input entry: shards across 8 NeuronCores, returns full output."""
    import time as _time
    in_maps = make_in_maps(inputs, NCORE_RUN, NLOC_RUN, T_RUN)
    last_err = None
    for attempt in range(4):
        try:
            if "runner" not in _STATE:
                nc = _STATE.get("nc")
                if nc is None:
                    nc = build(NCORE_RUN, T_RUN)
                    _STATE["nc"] = nc
                _STATE["runner"] = Runner(nc, NCORE_RUN)
            r = _STATE["runner"]
            r.prepare(in_maps)
            outs = r.run()
            res = r.results(outs)
            return assemble(res, NCORE_RUN, NLOC_RUN, T_RUN).astype(np.float32)
        except Exception as e:  # device hiccup: rebuild the jit and retry
            last_err = e
            _STATE.pop("runner", None)
            _time.sleep(15 * (attempt + 1))
    raise last_err

